# revision 1
# baseline (speedup 1.0000x reference)
"""Trainium2 Bass kernel for nn_AttentionModule_7146825580577.

Strategy (see spec sharding_hint): pure data parallel over the batch dim
(8192 rows -> 1024 rows per core, 8 cores), weights replicated.

Device math (per core), in feature-transposed layout (features on SBUF
partitions, batch on the free dim), fp32 data with float32r matmuls:

  - All LayerNorms whose input is an affine function of a previous
    activation use host-side column-centered weights, so mean(y) == 0 by
    construction and only sum(y^2) is needed on device (computed by a
    ones-vector matmul on the PE, reduced over partitions).
  - seq_len==1 MHA reduces to out_proj(v_proj(kv)); both projections are
    fused on the host into a single 512x512 effective matrix. The self-
    attention residual (x + sa(x)) is folded into a single matmul with
    weights I + Wv@Wo.
  - The cross-attention pair average (a+b)/2 is a single concat-matmul.
  - The n2 LayerNorm (after gating) is folded into the fus_W1 matmul:
    gamma scales fold into the weights, the per-sample mean correction is
    a rank-1 matmul term, betas fold into the bias.
  - 1/sqrt(var+eps) is computed on the vector engine with the int32 bit
    trick + Newton-Raphson iterations, on PE-transposed [128, k] stat
    tiles so each op touches only a tiny free dim.
  - Input hidden states / logits are transposed on the host (numpy) so no
    on-device transposes are needed; the output is produced transposed
    and transposed back on the host.
"""
import os
import sys

sys.path.insert(0, "/opt/trn_rl_repo")

import numpy as np

import concourse.bass as bass
import concourse.tile as tile
from concourse import bacc, mybir
from concourse.bass import ts
from concourse.bass_utils import run_bass_kernel_spmd
from concourse.masks import make_identity

D = 512
HID = 1024
B = 8192
NCORES = 8
BL = B // NCORES          # rows per core
NBT = BL // D             # batch tiles per core (2)
EPS = 1e-5
MAGIC = 0x5F3759DF
F32 = mybir.dt.float32
I32 = mybir.dt.int32
FS = [10, 6, 15]          # logit dims per stream
NR_ITERS = int(os.environ.get("KERNEL_NR_ITERS", "2"))
MM_DT = {
    "f32r": mybir.dt.float32r,
    "f32": mybir.dt.float32,
}[os.environ.get("KERNEL_MM_DTYPE", "f32r")]

F64 = np.float64


# --------------------------------------------------------------------------
# Host-side weight folding
# --------------------------------------------------------------------------

def _center_cols(W, b):
    W = np.asarray(W, F64)
    b = np.asarray(b, F64)
    return W - W.mean(axis=1, keepdims=True), b - b.mean()


def fold_weights(inp):
    g = lambda k: np.asarray(inp[k], dtype=F64)
    out = {}

    w_hp, b_hp = [], []
    for s in range(3):
        W, b = _center_cols(g("hp_W")[s], g("hp_b")[s])
        w_hp.append(W)
        b_hp.append(b)
    out["w_hp"] = np.stack(w_hp)
    out["b_hp"] = np.stack(b_hp)
    out["g_hp"], out["be_hp"] = g("hp_g"), g("hp_be")

    mhaW, mhab = g("mha_in_W"), g("mha_in_b")
    moW, mob = g("mha_out_W"), g("mha_out_b")
    Wv0, bv0 = mhaW[0][:, 2 * D:], mhab[0][2 * D:]
    Wr, br = _center_cols(np.eye(D) + Wv0 @ moW[0], bv0 @ moW[0] + mob[0])
    out["w_r"], out["b_r"] = Wr, br
    out["g_n1"], out["be_n1"] = g("n1_g"), g("n1_be")

    Wj, bj = [None] * 4, [None] * 4
    for j in (1, 2, 3):
        Wv, bv = mhaW[j][:, 2 * D:], mhab[j][2 * D:]
        Wj[j] = Wv @ moW[j]
        bj[j] = bv @ moW[j] + mob[j]
    # m_verb uses (inst_e @ W1, target_e @ W2); m_inst (verb @ W1, target @ W3);
    # m_target (verb @ W2, inst @ W3)
    mods = [(1, 2), (1, 3), (2, 3)]
    out["m_streams"] = [(1, 2), (0, 2), (0, 1)]
    w_m, b_m = [], []
    for s in range(3):
        ja, jb = mods[s]
        w_m.append(np.concatenate([0.5 * Wj[ja], 0.5 * Wj[jb]], axis=0))
        b_m.append(0.5 * (bj[ja] + bj[jb]))
    out["w_m"] = np.stack(w_m)
    out["b_m"] = np.stack(b_m)

    out["w_g"] = g("gate_W")
    out["b_g"] = g("gate_b")

    w_lp, b_lp = [], []
    for s, key in enumerate(["verb", "inst", "target"]):
        W, b = _center_cols(g(f"lp_W_{key}"), g(f"lp_b_{key}"))
        w_lp.append(W)
        b_lp.append(b)
    out["w_lp"] = w_lp
    out["b_lp"] = np.stack(b_lp)
    out["g_lp"], out["be_lp"] = g("lp_g"), g("lp_be")

    W1 = g("fus_W1")
    g2, be2 = g("n2_g"), g("n2_be")
    A1, negc = [], []
    bias_total = g("fus_b1").copy()
    for s in range(3):
        blk = W1[s * D:(s + 1) * D]
        A = g2[s][:, None] * blk
        c = blk.T @ g2[s]
        A1.append(A - A.mean(axis=1, keepdims=True))
        negc.append(-(c - c.mean()))
        bias_total += be2[s] @ blk
    L1 = []
    for s in range(3):
        off = 3 * D + s * (D // 2)
        blk = W1[off: off + D // 2]
        L1.append(blk - blk.mean(axis=1, keepdims=True))
    out["w_f1"] = np.stack(A1)
    out["negc_f1"] = np.stack(negc)
    out["w_f1l"] = np.stack(L1)
    out["b_f1"] = bias_total - bias_total.mean()
    out["g_f1"], out["be_f1"] = g("fus_g1"), g("fus_ge1")

    W2c, b2c = _center_cols(g("fus_W2"), g("fus_b2"))
    out["w_f2"], out["b_f2"] = W2c, b2c
    out["g_f2"], out["be_f2"] = g("fus_g2"), g("fus_ge2")
    return out


def _vec_pp(v, nk):
    """[.., nk*128] feature vector -> ACT per-partition layout [.., 128, nk]."""
    v = np.asarray(v, np.float32)
    return np.ascontiguousarray(v.reshape(v.shape[:-1] + (nk, 128)).swapaxes(-1, -2))


def device_arrays(fw):
    """Folded weights -> dict of fp32 arrays matching the DRAM tensor decls."""
    f32 = lambda v: np.ascontiguousarray(np.asarray(v, np.float32))
    dev = {}
    dev["w_hp"] = f32(fw["w_hp"].reshape(3, 8, 128, 512))
    dev["b_hp"] = _vec_pp(fw["b_hp"], 4)
    dev["w_r"] = f32(fw["w_r"].reshape(4, 128, 512))
    dev["b_r"] = _vec_pp(fw["b_r"], 4)
    dev["w_m"] = f32(fw["w_m"].reshape(3, 8, 128, 512))
    dev["b_m"] = _vec_pp(fw["b_m"], 4)
    dev["w_g"] = f32(fw["w_g"].reshape(3, 8, 128, 512))
    for s in range(3):
        dev[f"w_lp{s}"] = f32(fw["w_lp"][s])
    dev["b_lp"] = _vec_pp(fw["b_lp"], 2)
    dev["w_f1"] = f32(fw["w_f1"].reshape(3, 4, 128, 512))
    dev["w_f1l"] = f32(fw["w_f1l"].reshape(3, 2, 128, 512))
    dev["negc_f1"] = f32(fw["negc_f1"][None])
    dev["b_f1"] = _vec_pp(fw["b_f1"], 4)
    dev["w_f2"] = f32(fw["w_f2"].reshape(4, 128, 512))
    dev["b_f2"] = _vec_pp(fw["b_f2"], 4)
    for name in ("g_hp", "be_hp", "g_n1", "be_n1", "b_g"):
        dev[name] = _vec_pp(fw[name], 4)
    dev["g_lp"] = _vec_pp(fw["g_lp"], 2)
    dev["be_lp"] = _vec_pp(fw["be_lp"], 2)
    for name in ("g_f1", "be_f1", "g_f2", "be_f2"):
        dev[name] = _vec_pp(fw[name], 4)
    dev["ones_row"] = np.ones((1, 128), np.float32)
    dev["ones_col"] = np.ones((128, 1), np.float32)
    return dev


# --------------------------------------------------------------------------
# Device program
# --------------------------------------------------------------------------

class _Emit:
    def __init__(self, tc, io):
        self.tc = tc
        self.nc = tc.nc
        self.io = io
        self.ctx = None
        self.flip = 0

    def alt(self):
        """Alternate DVE / ACT for plain copies and squares."""
        self.flip ^= 1
        return self.flip

    def copy(self, out, in_, bias=None):
        """PSUM -> SBUF eviction, optionally adding a per-partition [128,1]
        bias column (the layer bias in transposed layout)."""
        nc = self.nc
        if self.alt():
            if bias is None:
                nc.vector.tensor_copy(out, in_)
            else:
                nc.vector.tensor_scalar_add(out, in_, bias)
        else:
            if bias is None:
                nc.scalar.activation(out, in_,
                                     mybir.ActivationFunctionType.Copy)
            else:
                nc.scalar.activation(out, in_,
                                     mybir.ActivationFunctionType.Identity,
                                     bias=bias)

    def square(self, out, in_sbuf, in_psum):
        """Square either from the evicted SBUF copy (DVE) or PSUM (ACT)."""
        nc = self.nc
        if self.alt():
            nc.vector.tensor_mul(out, in_sbuf, in_sbuf)
        else:
            nc.scalar.activation(out, in_psum,
                                 mybir.ActivationFunctionType.Square)


MF = MM_DT  # dtype of every tensor consumed by a matmul


def _rd(ap):
    return ap


DEBUG = bool(os.environ.get("KERNEL_DEBUG"))


def emit_program(tc, io):
    nc = tc.nc

    def dbg(name, tile_ap):
        if DEBUG and name in io:
            nc.sync.dma_start(io[name], tile_ap)
    from contextlib import ExitStack
    ctx = ExitStack()
    em = _Emit(tc, io)
    ACT = mybir.ActivationFunctionType

    # ---------------- pools ----------------
    P = lambda name, bufs, space="SBUF": ctx.enter_context(
        tc.tile_pool(name=name, bufs=bufs, space=space))
    const = P("const", 1)
    wpool = P("wchunk", 3)
    xpool = P("xchunk", 2)
    evp = P("ev", 10)
    sqp = P("sq", 2)
    zp = P("z", 2)
    yhp = P("yh", 1)
    ep = P("e", 3)
    mp = P("m", 2)
    sgp = P("sg", 1)
    qp = P("q", 1)
    tp = P("t", 2)
    ztp = P("zt", 3)
    lp_ = P("l", 3)
    hp_ = P("h", 1)
    op_ = P("o", 1)
    stp = P("stats_sb", 9)
    bcp = P("bc_sb", 2)
    ltp = P("lt", 1)
    mm_ps = P("mm_ps", 4, "PSUM")
    st_ps = P("st_ps", 4, "PSUM")

    # ---------------- constants / resident weights ----------------
    ident = const.tile([128, 128], F32)
    make_identity(nc, ident)
    ones_row = const.tile([1, 128], MF)
    nc.sync.dma_start(ones_row[:], io["ones_row"])
    ones_col = const.tile([128, 1], MF)
    nc.sync.dma_start(ones_col[:], io["ones_col"])

    def load(name, shape, rearr=None, dtype=F32):
        t = const.tile(shape, dtype, name=name)
        src = io[name]
        if rearr:
            src = src.rearrange(rearr)
        nc.sync.dma_start(t[:], src)
        return t

    b_hp = load("b_hp", [128, 3, 4], "s p c -> p s c")
    b_r = load("b_r", [128, 4])
    b_m = load("b_m", [128, 3, 4], "s p c -> p s c")
    b_lp = load("b_lp", [128, 3, 2], "s p c -> p s c")
    negc = load("negc_f1", [1, 3, 512], dtype=MF)
    b_f1 = load("b_f1", [128, 4])
    b_f2 = load("b_f2", [128, 4])
    g_hp = load("g_hp", [128, 3, 4], "s p c -> p s c")
    be_hp = load("be_hp", [128, 3, 4], "s p c -> p s c")
    g_n1 = load("g_n1", [128, 3, 4], "s p c -> p s c")
    be_n1 = load("be_n1", [128, 3, 4], "s p c -> p s c")
    b_g = load("b_g", [128, 3, 4], "s p c -> p s c")
    g_lp = load("g_lp", [128, 3, 2], "s p c -> p s c")
    be_lp = load("be_lp", [128, 3, 2], "s p c -> p s c")
    g_f1 = load("g_f1", [128, 4])
    be_f1 = load("be_f1", [128, 4])
    g_f2 = load("g_f2", [128, 4])
    be_f2 = load("be_f2", [128, 4])
    w_lp = [load(f"w_lp{s}", [FS[s], 256], dtype=MF) for s in range(3)]

    # ---------------- helpers ----------------
    def emit_istd(v_sb, k):
        """v_sb: [k,512] sbuf fp32 variances (+eps already added).
        Returns list of k istd row tiles [1,512] (MF), via PE-transposed
        Newton-Raphson rsqrt (int32 magic seed)."""
        vT = st_ps.tile([128, 4 * k], F32, name="vT", tag="stat_ps")
        for c in range(4):
            nc.tensor.transpose(vT[:, c * k:(c + 1) * k],
                                v_sb[0:k, ts(c, 128)], ident[0:k, 0:k])
        y = stp.tile([128, 4 * k], F32, name="nr_y", tag="ssb")
        t = stp.tile([128, 4 * k], F32, name="nr_t", tag="ssb")
        nc.vector.tensor_scalar(y[:].bitcast(I32), vT[:].bitcast(I32),
                                1, None, mybir.AluOpType.logical_shift_right)
        nc.vector.tensor_scalar(y[:].bitcast(I32), y[:].bitcast(I32),
                                -1, MAGIC, mybir.AluOpType.mult,
                                mybir.AluOpType.add)
        for _ in range(NR_ITERS):
            nc.vector.tensor_mul(t[:], y[:], y[:])
            nc.vector.tensor_mul(t[:], t[:], vT[:])
            nc.vector.tensor_scalar(t[:], t[:], -0.5, 1.5,
                                    mybir.AluOpType.mult, mybir.AluOpType.add)
            nc.vector.tensor_mul(y[:], y[:], t[:])
        rows = []
        for s in range(k):
            rT = st_ps.tile([1, 512], F32, name="rT", tag="stat_ps")
            for c in range(4):
                nc.tensor.transpose(rT[0:1, ts(c, 128)],
                                    y[:, c * k + s:c * k + s + 1], ident)
            istd = stp.tile([1, 512], MF, name="istd", tag="ssb")
            nc.vector.tensor_copy(istd[:], rT[:])
            rows.append(istd)
        return rows

    def bcast(row_ap):
        """[1,512] sbuf row -> [128,512] sbuf tile via GPSIMD."""
        bc = bcp.tile([128, 512], MF, name="bc")
        nc.gpsimd.partition_broadcast(bc[:], row_ap)
        return bc

    def emit_ln(ps_list, bias_cols=None):
        """Evict psum chunks to SBUF (adding the layer bias per partition)
        and accumulate sum(y^2) into a [1,512] psum row."""
        nch = len(ps_list)
        ev = []
        for c, ps in enumerate(ps_list):
            e = evp.tile([128, 512], F32, name="ev")
            em.copy(e[:], ps[:], None if bias_cols is None else bias_cols[c])
            ev.append(e)
        st = st_ps.tile([1, 512], F32, name="st", tag="stat_ps")
        for c in range(nch):
            sq = sqp.tile([128, 512], MF, name="sq")
            em.square(sq[:], ev[c][:], ev[c][:])
            nc.tensor.matmul(st[:], ones_col[:], sq[:],
                             start=(c == 0), stop=(c == nch - 1))
        return ev, st

    def ln_finish(ev, st, gam, bet, func, out_tile, dim=D):
        v = stp.tile([1, 512], F32, name="v", tag="ssb")
        nc.vector.tensor_scalar(v[0:1, :], st[:], 1.0 / dim, EPS,
                                mybir.AluOpType.mult, mybir.AluOpType.add)
        istd = emit_istd(v, 1)[0]
        bc = bcast(istd[:])
        for c, e in enumerate(ev):
            z = zp.tile([128, 512], F32, name="z")
            nc.vector.tensor_mul(z[:], e[:], bc[:])
            nc.scalar.activation(out_tile[:, c, :], z[:], func,
                                 bias=bet[:, c:c + 1], scale=gam[:, c:c + 1])

    def mm_group(n_m, srcs, bias_cols):
        """Emit an accumulating matmul group. srcs = list of (lhsT_fn, rhs)
        k-chunks; returns (ev, st) after evict+square+stats."""
        ps_list = [mm_ps.tile([128, 512], F32, name="mm") for _ in range(n_m)]
        last = len(srcs) - 1
        for ci, (lhsT_fn, rhs) in enumerate(srcs):
            for m in range(n_m):
                nc.tensor.matmul(ps_list[m][:], lhsT_fn(m), rhs,
                                 start=(ci == 0), stop=(ci == last))
        return ps_list

    # ---------------- main ----------------
    pend = []

    def flush(n=None):
        cnt = len(pend) if n is None else n
        for _ in range(cnt):
            if pend:
                pend.pop(0)()

    def wchunk(dram_ap):
        wc = wpool.tile([128, 512], MF, name="wc", tag="wc1")
        nc.sync.dma_start(wc[:], dram_ap)
        return wc

    def wpair(dram_pair_ap):
        """Load two [128,512] k-chunks in one DMA -> [128,2,512] tile."""
        wc = wpool.tile([128, 2, 512], MF, name="wcp", tag="wcp")
        nc.sync.dma_start(wc[:], dram_pair_ap.rearrange("c p n -> p c n"))
        return wc

    def pair_srcs(dram_4d, nk, rhs_fn):
        srcs = []
        for c0 in range(0, nk, 2):
            wc = wpair(dram_4d[c0:c0 + 2])
            for cc in range(2):
                srcs.append((lambda m, wc=wc, cc=cc: wc[:, cc, ts(m, 128)],
                             rhs_fn(c0 + cc)))
        return srcs

    for bt in range(NBT):
        bsl = ts(bt, 512)
        l_tiles = [None] * 3
        e_tiles = [None] * 3
        m_tiles = [None] * 3
        zt_tiles = [None] * 3
        w_rows = [None] * 3
        yh_tiles = [None] * 3

        # ---- lp matmuls (tiny) ----
        def emit_lp_mm(s):
            lt = ltp.tile([FS[s], 512], F32, name="lt", tag="lt")
            nc.sync.dma_start(lt[:], io[f"lT{s}"][:, bsl])
            lsg = ltp.tile([FS[s], 512], MF, name="lsg", tag="lsg")
            nc.scalar.activation(lsg[:], lt[:], ACT.Sigmoid)
            ps_list = [mm_ps.tile([128, 512], F32, name="mm") for _ in range(2)]
            for m in range(2):
                nc.tensor.matmul(ps_list[m][:], w_lp[s][:, ts(m, 128)],
                                 lsg[:], start=True, stop=True)
            ev, st = emit_ln(ps_list, [b_lp[:, s, c:c + 1] for c in range(2)])

            def fin(s=s, ev=ev, st=st):
                l_sb = lp_.tile([128, 2, 512], MF, name="l_sb")
                ln_finish(ev, st, g_lp[:, s], be_lp[:, s], ACT.Gelu, l_sb,
                          dim=D // 2)
                l_tiles[s] = l_sb
            pend.append(fin)

        def emit_hp_mm(s):
            xcs = []
            for c0 in range(0, 8, 2):
                xc = xpool.tile([128, 2, 512], MF, name="xc")
                nc.sync.dma_start(
                    xc[:], io[f"xT{s}"][ts(c0 // 2, 256), bsl].rearrange(
                        "(c p) b -> p c b", p=128))
                xcs.append(xc)
            srcs = pair_srcs(io["w_hp"][s], 8,
                             lambda c: xcs[c // 2][:, c % 2, :])
            ps_list = mm_group(4, srcs, None)
            ev, st = emit_ln(ps_list, [b_hp[:, s, c:c + 1] for c in range(4)])

            def fin(s=s, ev=ev, st=st):
                yh = yhp.tile([128, 4, 512], MF, name="yh")
                ln_finish(ev, st, g_hp[:, s], be_hp[:, s], ACT.Gelu, yh)
                yh_tiles[s] = yh
            pend.append(fin)

        def emit_r_mm(s):
            yh = yh_tiles[s]
            srcs = pair_srcs(io["w_r"], 4, lambda c: yh[:, c, :])
            ps_list = mm_group(4, srcs, None)
            ev, st = emit_ln(ps_list, [b_r[:, c:c + 1] for c in range(4)])

            def fin(s=s, ev=ev, st=st):
                e_sb = ep.tile([128, 4, 512], MF, name="e_sb")
                ln_finish(ev, st, g_n1[:, s], be_n1[:, s], ACT.Identity, e_sb)
                e_tiles[s] = e_sb
            pend.append(fin)

        m_streams = [(1, 2), (0, 2), (0, 1)]

        def emit_m_mm(s):
            sa, sb = m_streams[s]
            srcs = pair_srcs(io["w_m"][s], 8,
                             lambda ci: (e_tiles[sa][:, ci, :] if ci < 4
                                         else e_tiles[sb][:, ci - 4, :]))
            ps_list = mm_group(4, srcs, None)
            m_sb = mp.tile([128, 4, 512], MF, name="m_sb")
            for c in range(4):
                em.copy(m_sb[:, c, :], ps_list[c][:], b_m[:, s, c:c + 1])
            m_tiles[s] = m_sb

        def emit_gate_fuse(s):
            srcs = pair_srcs(io["w_g"][s], 8,
                             lambda ci: (e_tiles[s][:, ci, :] if ci < 4
                                         else m_tiles[s][:, ci - 4, :]))
            ps_list = mm_group(4, srcs, None)
            t_sb = tp.tile([128, 4, 512], MF, name="t_sb")
            for c in range(4):
                sg = sgp.tile([128, 512], F32, name="sg")
                nc.scalar.activation(sg[:], ps_list[c][:], ACT.Sigmoid,
                                     bias=b_g[:, s, c:c + 1])
                q = qp.tile([128, 512], F32, name="q")
                nc.vector.tensor_mul(q[:], sg[:], m_tiles[s][:, c, :])
                nc.vector.tensor_add(t_sb[:, c, :], e_tiles[s][:, c, :], q[:])
            st_sum = st_ps.tile([1, 512], F32, name="st_sum", tag="stat_ps")
            st_sq = st_ps.tile([1, 512], F32, name="st_sq", tag="stat_ps")
            for c in range(4):
                nc.tensor.matmul(st_sum[:], ones_col[:], t_sb[:, c, :],
                                 start=(c == 0), stop=(c == 3))
            for c in range(4):
                sq = sqp.tile([128, 512], MF, name="sq")
                em.square(sq[:], t_sb[:, c, :], t_sb[:, c, :])
                nc.tensor.matmul(st_sq[:], ones_col[:], sq[:],
                                 start=(c == 0), stop=(c == 3))

            def fin(s=s, t_sb=t_sb, st_sum=st_sum, st_sq=st_sq):
                mu = stp.tile([1, 512], F32, name="mu", tag="ssb")
                nc.vector.tensor_scalar_mul(mu[:], st_sum[:], 1.0 / D)
                ev2 = stp.tile([1, 512], F32, name="ev2", tag="ssb")
                nc.vector.tensor_scalar(ev2[:], st_sq[:], 1.0 / D, EPS,
                                        mybir.AluOpType.mult,
                                        mybir.AluOpType.add)
                v = stp.tile([1, 512], F32, name="v", tag="ssb")
                nc.vector.tensor_mul(v[:], mu[:], mu[:])
                nc.vector.tensor_sub(v[:], ev2[:], v[:])
                istd = emit_istd(v, 1)[0]
                w_row = stp.tile([1, 512], MF, name="w_row", tag="ssb")
                nc.vector.tensor_mul(w_row[:], mu[:], istd[:])
                w_rows[s] = w_row
                bc = bcast(istd[:])
                zt = ztp.tile([128, 4, 512], MF, name="zt")
                for c in range(4):
                    nc.vector.tensor_mul(zt[:, c, :], t_sb[:, c, :], bc[:])
                zt_tiles[s] = zt
            pend.append(fin)

        # ---------- emission schedule (software pipelined) ----------
        emit_lp_mm(0)
        emit_lp_mm(1)
        emit_lp_mm(2)
        emit_hp_mm(0)          # hp0 matmuls cover lp NR chains
        flush(2)               # lp0, lp1 fins
        emit_hp_mm(1)
        flush(2)               # lp2 fin + hp0 fin (covered by hp1 matmuls)
        emit_r_mm(0)
        flush(1)               # hp1 fin (covered by r'0/hp1 matmuls)
        emit_hp_mm(2)
        emit_r_mm(1)
        flush(1)               # n1_0 fin -> e0
        flush(1)               # hp2 fin -> yh2
        emit_r_mm(2)
        flush(1)               # n1_1 fin -> e1
        emit_m_mm(2)           # m_target needs e0,e1
        flush(1)               # n1_2 fin -> e2
        emit_gate_fuse(2)
        emit_m_mm(1)           # m_inst needs e0,e2
        flush(1)               # n2_2 fin -> zt2 (covered by m1 matmuls)
        emit_gate_fuse(1)
        emit_m_mm(0)           # m_verb needs e1,e2
        flush(1)               # n2_1 fin -> zt1 (covered by m0 matmuls)
        emit_gate_fuse(0)
        flush(1)               # n2_0 fin -> zt0 (covered by fus1 l/zt2/zt1)

        # ---- fus1: order k-chunks so zt0 (finished last) is consumed last
        srcs = []
        for s in range(3):
            srcs += pair_srcs(io["w_f1l"][s], 2,
                              lambda c, s=s: l_tiles[s][:, c, :])
        for s in (2, 1, 0):
            srcs += pair_srcs(io["w_f1"][s], 4,
                              lambda c, s=s: zt_tiles[s][:, c, :])
        for s in (2, 1, 0):
            srcs.append((lambda m, s=s: negc[0:1, s, ts(m, 128)],
                         w_rows[s][:]))
        ps_list = mm_group(4, srcs, None)
        ev, st = emit_ln(ps_list, [b_f1[:, c:c + 1] for c in range(4)])

        def fin_f1(ev=ev, st=st):
            h_sb = hp_.tile([128, 4, 512], MF, name="h_sb")
            ln_finish(ev, st, g_f1, be_f1, ACT.Gelu, h_sb)
            fin_f1.h = h_sb
        pend.append(fin_f1)
        flush(1)

        # ---- fus2
        h_sb = fin_f1.h
        srcs = pair_srcs(io["w_f2"], 4, lambda c: h_sb[:, c, :])
        ps_list = mm_group(4, srcs, None)
        ev, st = emit_ln(ps_list, [b_f2[:, c:c + 1] for c in range(4)])

        def fin_f2(ev=ev, st=st, bsl=bsl):
            o_sb = op_.tile([128, 4, 512], F32, name="o_sb")
            ln_finish(ev, st, g_f2, be_f2, ACT.Identity, o_sb)
            nc.sync.dma_start(
                io["outT"].rearrange("(c p) b -> p c b", p=128)[:, :, bsl],
                o_sb[:])
        pend.append(fin_f2)
        flush(1)

    flush()
    ctx.close()


def build_program():
    nc = bacc.Bacc("TRN2", target_bir_lowering=False, debug=False,
                   num_devices=NCORES)
    io = {}

    def din(name, shape, dtype=F32):
        io[name] = nc.dram_tensor(name, list(shape), dtype,
                                  kind="ExternalInput").ap()

    for s in range(3):
        din(f"xT{s}", (HID, BL), dtype=MM_DT)
        din(f"lT{s}", (FS[s], BL))
    din("w_hp", (3, 8, 128, 512), dtype=MM_DT)
    din("b_hp", (3, 128, 4))
    din("w_r", (4, 128, 512), dtype=MM_DT)
    din("b_r", (128, 4))
    din("w_m", (3, 8, 128, 512), dtype=MM_DT)
    din("b_m", (3, 128, 4))
    din("w_g", (3, 8, 128, 512), dtype=MM_DT)
    for s in range(3):
        din(f"w_lp{s}", (FS[s], 256), dtype=MM_DT)
    din("b_lp", (3, 128, 2))
    din("w_f1", (3, 4, 128, 512), dtype=MM_DT)
    din("w_f1l", (3, 2, 128, 512), dtype=MM_DT)
    din("negc_f1", (1, 3, 512), dtype=MM_DT)
    din("b_f1", (128, 4))
    din("w_f2", (4, 128, 512), dtype=MM_DT)
    din("b_f2", (128, 4))
    for name in ("g_hp", "be_hp", "g_n1", "be_n1", "b_g"):
        din(name, (3, 128, 4))
    for name in ("g_lp", "be_lp"):
        din(name, (3, 128, 2))
    for name in ("g_f1", "be_f1", "g_f2", "be_f2"):
        din(name, (128, 4))
    din("ones_row", (1, 128), dtype=MM_DT)
    din("ones_col", (128, 1), dtype=MM_DT)
    io["outT"] = nc.dram_tensor("outT", [D, BL], F32,
                                kind="ExternalOutput").ap()
    if os.environ.get("KERNEL_DEBUG"):
        for s in range(3):
            for nm, shp in [(f"dbg_istd_hp{s}", [1, 512]),
                            (f"dbg_yh{s}", [128, 4, 512]),
                            (f"dbg_e{s}", [128, 4, 512]),
                            (f"dbg_l{s}", [128, 2, 512]),
                            (f"dbg_m{s}", [128, 4, 512]),
                            (f"dbg_t{s}", [128, 4, 512]),
                            (f"dbg_w{s}", [1, 512])]:
                io[nm] = nc.dram_tensor(nm, shp, F32,
                                        kind="ExternalOutput").ap()
        io["dbg_h"] = nc.dram_tensor("dbg_h", [128, 4, 512], F32,
                                     kind="ExternalOutput").ap()

    with tile.TileContext(nc) as tc:
        emit_program(tc, io)
    nc.compile()
    return nc


def make_in_maps(inputs):
    fw = fold_weights(inputs)
    dev = device_arrays(fw)
    hidden = [np.asarray(inputs["verb_hidden"], np.float32),
              np.asarray(inputs["inst_hidden"], np.float32),
              np.asarray(inputs["target_hidden"], np.float32)]
    logits = [np.asarray(inputs["verb_logits"], np.float32),
              np.asarray(inputs["inst_logits"], np.float32),
              np.asarray(inputs["target_logits"], np.float32)]
    in_maps = []
    for core in range(NCORES):
        rows = slice(core * BL, (core + 1) * BL)
        m = dict(dev)
        for s in range(3):
            m[f"xT{s}"] = np.ascontiguousarray(hidden[s][rows].T)
            m[f"lT{s}"] = np.ascontiguousarray(logits[s][rows].T)
        in_maps.append(m)
    return in_maps


_NC_CACHE = None


def _run(inputs, **spmd_kwargs):
    global _NC_CACHE
    if _NC_CACHE is None:
        _NC_CACHE = build_program()
    nc = _NC_CACHE
    in_maps = make_in_maps(inputs)
    res = run_bass_kernel_spmd(nc, in_maps, list(range(NCORES)),
                               **spmd_kwargs)
    out = np.empty((B, D), dtype=np.float32)
    for core in range(NCORES):
        out[core * BL:(core + 1) * BL] = res.results[core]["outT"].T
    return out, res


def kernel(**inputs) -> np.ndarray:
    return _run(inputs)[0]


def kernel_profiled(inputs, tmpdir=None):
    """Returns (out, BassKernelResults) with an NTFF-based profile."""
    return _run(inputs, trace=True, tmpdir=tmpdir)



# revision 5
# speedup vs baseline: 1.5642x; 1.5642x over previous
"""Trainium2 Bass kernel for nn_AttentionModule_7146825580577.

Strategy: pure data parallel over the batch dim (8192 rows -> 1024 rows
per core, 8 cores), weights replicated.

Device math (per core), feature-transposed layout (features on SBUF
partitions, batch on the free dim), bf16 matmul operands with fp32 PSUM
accumulation:

  - LayerNorms over affine-of-activation inputs use host-side
    column-centered weights, so mean(y) == 0 by construction and only
    sum(y^2) is needed (ones-vector matmul on the PE).
  - seq_len==1 MHA reduces to out_proj(v_proj(kv)); fused on the host
    into single 512x512 matrices; self-attention residual folded as
    I + Wv@Wo.
  - The n2 LayerNorm (post-gating) is folded into the fus_W1 matmul:
    gamma scales fold into weights, the per-sample mean correction is a
    rank-1 matmul term (k=3 packed), betas fold into the bias.
  - 1/sqrt(var) via DVE reciprocal_approx_fast + ACT Sqrt on [1,512]
    stat rows (no PE transposes, no NR loop); istd broadcast across
    partitions on GPSIMD.
  - The two 512-column batch tiles are processed in lockstep per
    (stage, stream) group so each weight chunk is DMA'd from HBM once
    and consumed by both tiles back-to-back; LN chains of group k
    execute under the matmuls of group k+1, keeping the PE dense (and
    the HAM clock-gate warm).
"""
import os
import sys

sys.path.insert(0, "/opt/trn_rl_repo")

import numpy as np
import ml_dtypes

import concourse.bass as bass
import concourse.tile as tile
from concourse import bacc, mybir
from concourse.bass import ts
from concourse.bass_utils import run_bass_kernel_spmd

D = 512
HID = 1024
B = 8192
NCORES = 8
BL = B // NCORES          # rows per core
NBT = BL // D             # batch tiles per core (2)
EPS = 1e-5
F32 = mybir.dt.float32
BF = mybir.dt.bfloat16
FS = [10, 6, 15]          # logit dims per stream
F64 = np.float64
BF_NP = ml_dtypes.bfloat16


# --------------------------------------------------------------------------
# Host-side weight folding (float64)
# --------------------------------------------------------------------------

def _center_cols(W, b):
    W = np.asarray(W, F64)
    b = np.asarray(b, F64)
    return W - W.mean(axis=1, keepdims=True), b - b.mean()


def fold_weights(inp):
    g = lambda k: np.asarray(inp[k], dtype=F64)
    out = {}

    w_hp, b_hp = [], []
    for s in range(3):
        W, b = _center_cols(g("hp_W")[s], g("hp_b")[s])
        w_hp.append(W)
        b_hp.append(b)
    out["w_hp"] = np.stack(w_hp)
    out["b_hp"] = np.stack(b_hp)
    out["g_hp"], out["be_hp"] = g("hp_g"), g("hp_be")

    mhaW, mhab = g("mha_in_W"), g("mha_in_b")
    moW, mob = g("mha_out_W"), g("mha_out_b")
    Wv0, bv0 = mhaW[0][:, 2 * D:], mhab[0][2 * D:]
    Wr, br = _center_cols(np.eye(D) + Wv0 @ moW[0], bv0 @ moW[0] + mob[0])
    out["w_r"], out["b_r"] = Wr, br
    out["g_n1"], out["be_n1"] = g("n1_g"), g("n1_be")

    Wj, bj = [None] * 4, [None] * 4
    for j in (1, 2, 3):
        Wv, bv = mhaW[j][:, 2 * D:], mhab[j][2 * D:]
        Wj[j] = Wv @ moW[j]
        bj[j] = bv @ moW[j] + mob[j]
    # m_verb uses (inst_e, target_e); m_inst (verb, target); m_target (verb, inst)
    mods = [(1, 2), (1, 3), (2, 3)]
    w_m, b_m = [], []
    for s in range(3):
        ja, jb = mods[s]
        w_m.append(np.concatenate([0.5 * Wj[ja], 0.5 * Wj[jb]], axis=0))
        b_m.append(0.5 * (bj[ja] + bj[jb]))
    out["w_m"] = np.stack(w_m)
    out["b_m"] = np.stack(b_m)

    out["w_g"] = g("gate_W")
    out["b_g"] = g("gate_b")

    w_lp, b_lp = [], []
    for s, key in enumerate(["verb", "inst", "target"]):
        W, b = _center_cols(g(f"lp_W_{key}"), g(f"lp_b_{key}"))
        w_lp.append(W)
        b_lp.append(b)
    out["w_lp"] = w_lp
    out["b_lp"] = np.stack(b_lp)
    out["g_lp"], out["be_lp"] = g("lp_g"), g("lp_be")

    W1 = g("fus_W1")
    g2, be2 = g("n2_g"), g("n2_be")
    A1, negc = [], []
    bias_total = g("fus_b1").copy()
    for s in range(3):
        blk = W1[s * D:(s + 1) * D]
        A = g2[s][:, None] * blk
        c = blk.T @ g2[s]
        A1.append(A - A.mean(axis=1, keepdims=True))
        negc.append(-(c - c.mean()))
        bias_total += be2[s] @ blk
    L1 = []
    for s in range(3):
        off = 3 * D + s * (D // 2)
        blk = W1[off: off + D // 2]
        L1.append(blk - blk.mean(axis=1, keepdims=True))
    out["w_f1"] = np.stack(A1)
    out["negc_f1"] = np.stack(negc)
    out["w_f1l"] = np.stack(L1)
    out["b_f1"] = bias_total - bias_total.mean()
    out["g_f1"], out["be_f1"] = g("fus_g1"), g("fus_ge1")

    W2c, b2c = _center_cols(g("fus_W2"), g("fus_b2"))
    out["w_f2"], out["b_f2"] = W2c, b2c
    out["g_f2"], out["be_f2"] = g("fus_g2"), g("fus_ge2")
    return out


def _vec_pp(v, nk):
    """[.., nk*128] feature vector -> per-partition layout [.., 128, nk]."""
    v = np.asarray(v, np.float32)
    return np.ascontiguousarray(v.reshape(v.shape[:-1] + (nk, 128)).swapaxes(-1, -2))


def device_arrays(fw):
    f32 = lambda v: np.ascontiguousarray(np.asarray(v, np.float32))
    bf = lambda v: np.ascontiguousarray(
        np.asarray(v, np.float32).astype(BF_NP))
    dev = {}
    dev["w_hp"] = bf(fw["w_hp"].reshape(3, 8, 128, 512))
    dev["b_hp"] = _vec_pp(fw["b_hp"], 4)
    dev["w_r"] = bf(fw["w_r"].reshape(4, 128, 512))
    dev["b_r"] = _vec_pp(fw["b_r"], 4)
    dev["w_m"] = bf(fw["w_m"].reshape(3, 8, 128, 512))
    dev["b_m"] = _vec_pp(fw["b_m"], 4)
    dev["w_g"] = bf(fw["w_g"].reshape(3, 8, 128, 512))
    for s in range(3):
        dev[f"w_lp{s}"] = bf(fw["w_lp"][s])
    dev["b_lp"] = _vec_pp(fw["b_lp"], 2)
    dev["w_f1"] = bf(fw["w_f1"].reshape(3, 4, 128, 512))
    dev["w_f1l"] = bf(fw["w_f1l"].reshape(3, 2, 128, 512))
    dev["negc_f1"] = bf(fw["negc_f1"][None])     # [1, 3, 512]
    dev["b_f1"] = _vec_pp(fw["b_f1"], 4)
    dev["w_f2"] = bf(fw["w_f2"].reshape(4, 128, 512))
    dev["b_f2"] = _vec_pp(fw["b_f2"], 4)
    for name in ("g_hp", "be_hp", "g_n1", "be_n1", "b_g"):
        dev[name] = _vec_pp(fw[name], 4)
    dev["g_lp"] = _vec_pp(fw["g_lp"], 2)
    dev["be_lp"] = _vec_pp(fw["be_lp"], 2)
    for name in ("g_f1", "be_f1", "g_f2", "be_f2"):
        dev[name] = _vec_pp(fw[name], 4)
    dev["ones_col"] = np.ones((128, 1), BF_NP)
    dev["eps_lhs"] = np.full((1, 1), (D // 2) * EPS, BF_NP)
    dev["one_row"] = np.ones((1, 512), BF_NP)
    return dev


# --------------------------------------------------------------------------
# Device program
# --------------------------------------------------------------------------

def emit_program(tc, io):
    nc = tc.nc
    ACT = mybir.ActivationFunctionType
    ALU = mybir.AluOpType
    from contextlib import ExitStack
    ctx = ExitStack()

    P = lambda name, bufs, space="SBUF": ctx.enter_context(
        tc.tile_pool(name=name, bufs=bufs, space=space))
    const = P("const", 1)
    wpool = P("w", 10)
    xpool = P("x", 8)
    lpool = P("l", 6)
    big = P("big", 15)
    tpool = P("t", 3)
    mp = P("m", 9)
    evp = P("ev", 9)
    sqp = P("sq", 6)
    zp = P("z", 6)
    qp = P("q", 4)
    sgp = P("sg", 4)
    bcp = P("bc", 6)
    op_ = P("o", 4)
    rowf = P("rowf", 6)
    rowi = P("rowi", 3)
    wrp = P("wr", 6)
    ltp = P("lt", 4)
    mm_ps = P("mm_ps", 6, "PSUM")
    st_ps = P("st_ps", 2, "PSUM")

    # ---------------- constants ----------------
    def load(name, shape, rearr=None, dtype=F32):
        t = const.tile(shape, dtype, name=name)
        src = io[name]
        if rearr:
            src = src.rearrange(rearr)
        nc.sync.dma_start(t[:], src)
        return t

    ones_col = load("ones_col", [128, 1], dtype=BF)
    eps_lhs = load("eps_lhs", [1, 1], dtype=BF)
    one_row = load("one_row", [1, 512], dtype=BF)
    negc3 = load("negc_f1", [1, 3, 512], dtype=BF)
    b_hp = load("b_hp", [128, 3, 4], "s p c -> p s c")
    b_r = load("b_r", [128, 4])
    b_m = load("b_m", [128, 3, 4], "s p c -> p s c")
    b_lp = load("b_lp", [128, 3, 2], "s p c -> p s c")
    b_f1 = load("b_f1", [128, 4])
    b_f2 = load("b_f2", [128, 4])
    g_hp = load("g_hp", [128, 3, 4], "s p c -> p s c")
    be_hp = load("be_hp", [128, 3, 4], "s p c -> p s c")
    g_n1 = load("g_n1", [128, 3, 4], "s p c -> p s c")
    be_n1 = load("be_n1", [128, 3, 4], "s p c -> p s c")
    b_g = load("b_g", [128, 3, 4], "s p c -> p s c")
    g_lp = load("g_lp", [128, 3, 2], "s p c -> p s c")
    be_lp = load("be_lp", [128, 3, 2], "s p c -> p s c")
    g_f1 = load("g_f1", [128, 4])
    be_f1 = load("be_f1", [128, 4])
    g_f2 = load("g_f2", [128, 4])
    be_f2 = load("be_f2", [128, 4])
    w_lp = [load(f"w_lp{s}", [FS[s], 256], dtype=BF) for s in range(3)]

    # ---------------- helpers ----------------
    pend = []

    def flush(n=None):
        cnt = len(pend) if n is None else n
        for _ in range(cnt):
            if pend:
                pend.pop(0)()

    def wpair(dram_pair_ap):
        wc = wpool.tile([128, 2, 512], BF, name="wc")
        nc.sync.dma_start(wc[:], dram_pair_ap.rearrange("c p n -> p c n"))
        return wc

    def load_pairs(dram_4d, nk):
        """DMA nk [128,512] weight k-chunks (as nk/2 pair tiles); returns
        per-chunk lhsT accessor fns."""
        fns = []
        for c0 in range(0, nk, 2):
            wc = wpair(dram_4d[c0:c0 + 2])
            for cc in range(2):
                fns.append(lambda m, wc=wc, cc=cc: wc[:, cc, ts(m, 128)])
        return fns

    def emit_mms(lhs_fns, rhs_fn, nm=4):
        ps = [mm_ps.tile([128, 512], F32, name="mm") for _ in range(nm)]
        last = len(lhs_fns) - 1
        for ci, lf in enumerate(lhs_fns):
            rhs = rhs_fn(ci)
            for m in range(nm):
                nc.tensor.matmul(ps[m][:], lf(m), rhs,
                                 start=(ci == 0), stop=(ci == last))
        return ps

    def evict_sq(ps_list, bias_cols, do_sq=True, pool=None):
        ev, sq = [], []
        for c, psx in enumerate(ps_list):
            e = (pool or evp).tile([128, 512], BF, name="evt")
            nc.scalar.activation(e[:], psx[:], ACT.Identity, bias=bias_cols[c])
            ev.append(e)
            if do_sq:
                s = sqp.tile([128, 512], BF, name="sqt")
                nc.vector.tensor_mul(s[:], e[:], e[:])
                sq.append(s)
        return ev, sq

    def stats(sq_list, add_eps=False):
        st = st_ps.tile([1, 512], F32, name="st", tag="stps")
        n = len(sq_list) + (1 if add_eps else 0)
        for c, s in enumerate(sq_list):
            nc.tensor.matmul(st[:], ones_col[:], s[:],
                             start=(c == 0), stop=(c == n - 1))
        if add_eps:
            nc.tensor.matmul(st[:], eps_lhs[:], one_row[:],
                             start=False, stop=True)
        return st

    def istd_chain(st, dim):
        rec = rowf.tile([1, 512], F32, name="rec", tag="rowf")
        nc.vector.reciprocal_approx_fast(rec[:], st[:])
        ist = rowi.tile([1, 512], BF, name="ist", tag="rowi")
        nc.scalar.activation(ist[:], rec[:], ACT.Sqrt, scale=float(dim))
        bc = bcp.tile([128, 512], BF, name="bct")
        nc.gpsimd.partition_broadcast(bc[:], ist[0:1, :])
        return bc

    def ln_finish(ev, st, dim, gam, bet, func, out_tile):
        bc = istd_chain(st, dim)
        for c, e in enumerate(ev):
            z = zp.tile([128, 512], BF, name="zt")
            nc.vector.tensor_mul(z[:], e[:], bc[:])
            nc.scalar.activation(out_tile[:, c, :], z[:], func,
                                 bias=bet[:, c:c + 1], scale=gam[:, c:c + 1])

    # ---------------- per-stage state ----------------
    yh = [[None] * 2 for _ in range(3)]
    e_ = [[None] * 2 for _ in range(3)]
    m_ = [[None] * 2 for _ in range(3)]
    zt = [[None] * 2 for _ in range(3)]
    l_ = [[None] * 2 for _ in range(3)]
    h_ = [None] * 2
    wrow = [[None] * 3 for _ in range(2)]
    hpw = [None] * 3
    mw = [None] * 3
    gw = [None] * 3
    rw = [None]
    f1w = [None]
    f2w = [None]

    def lp_group(s, bt):
        bsl = ts(bt, 512)
        lt = ltp.tile([FS[s], 512], F32, name="ltt", tag="ltt")
        nc.sync.dma_start(lt[:], io[f"lT{s}"][:, bsl])
        lsg = ltp.tile([FS[s], 512], BF, name="lsg", tag="ltt")
        nc.scalar.activation(lsg[:], lt[:], ACT.Sigmoid)
        ps = [mm_ps.tile([128, 512], F32, name="mm") for _ in range(2)]
        for m in range(2):
            nc.tensor.matmul(ps[m][:], w_lp[s][:, ts(m, 128)], lsg[:],
                             start=True, stop=True)
        ev, sq = evict_sq(ps, [b_lp[:, s, c:c + 1] for c in range(2)])

        def fin(s=s, bt=bt, ev=ev, sq=sq):
            st = stats(sq, add_eps=True)
            l_sb = lpool.tile([128, 2, 512], BF, name="l_sb")
            ln_finish(ev, st, D // 2, g_lp[:, s], be_lp[:, s], ACT.Gelu, l_sb)
            l_[s][bt] = l_sb
        pend.append(fin)

    def hp_group(s, bt):
        bsl = ts(bt, 512)
        xcs = []
        for c0 in range(0, 8, 2):
            xc = xpool.tile([128, 2, 512], BF, name="xc")
            nc.sync.dma_start(
                xc[:], io[f"xT{s}"][ts(c0 // 2, 256), bsl].rearrange(
                    "(c p) b -> p c b", p=128))
            xcs.append(xc)
        if bt == 0:
            hpw[s] = load_pairs(io["w_hp"][s], 8)
        ps = emit_mms(hpw[s], lambda c: xcs[c // 2][:, c % 2, :])
        ev, sq = evict_sq(ps, [b_hp[:, s, c:c + 1] for c in range(4)])

        def fin(s=s, bt=bt, ev=ev, sq=sq):
            st = stats(sq)
            y_sb = big.tile([128, 4, 512], BF, name="big_sb")
            ln_finish(ev, st, D, g_hp[:, s], be_hp[:, s], ACT.Gelu, y_sb)
            yh[s][bt] = y_sb
        pend.append(fin)

    def r_group(s, bt):
        if rw[0] is None:
            rw[0] = load_pairs(io["w_r"], 4)
        ps = emit_mms(rw[0], lambda c: yh[s][bt][:, c, :])
        ev, sq = evict_sq(ps, [b_r[:, c:c + 1] for c in range(4)])

        def fin(s=s, bt=bt, ev=ev, sq=sq):
            st = stats(sq)
            e_sb = big.tile([128, 4, 512], BF, name="big_sb")
            ln_finish(ev, st, D, g_n1[:, s], be_n1[:, s], ACT.Identity, e_sb)
            e_[s][bt] = e_sb
        pend.append(fin)

    m_streams = [(1, 2), (0, 2), (0, 1)]

    def m_group(s, bt):
        if bt == 0:
            mw[s] = load_pairs(io["w_m"][s], 8)
        sa, sb = m_streams[s]
        ps = emit_mms(mw[s], lambda c: (e_[sa][bt][:, c, :] if c < 4
                                        else e_[sb][bt][:, c - 4, :]))
        ev, _ = evict_sq(ps, [b_m[:, s, c:c + 1] for c in range(4)],
                         do_sq=False, pool=mp)
        m_[s][bt] = ev

    def g_group(s, bt):
        if bt == 0:
            gw[s] = load_pairs(io["w_g"][s], 8)
        ps = emit_mms(gw[s], lambda c: (e_[s][bt][:, c, :] if c < 4
                                        else m_[s][bt][c - 4][:]))
        t_sb = tpool.tile([128, 4, 512], BF, name="t_sb")
        sqs = []
        for c in range(4):
            sg = sgp.tile([128, 512], BF, name="sgt")
            nc.scalar.activation(sg[:], ps[c][:], ACT.Sigmoid,
                                 bias=b_g[:, s, c:c + 1])
            q = qp.tile([128, 512], BF, name="qt")
            nc.vector.tensor_mul(q[:], sg[:], m_[s][bt][c][:])
            nc.vector.tensor_add(t_sb[:, c, :], e_[s][bt][:, c, :], q[:])
            sqc = sqp.tile([128, 512], BF, name="sqt")
            nc.vector.tensor_mul(sqc[:], t_sb[:, c, :], t_sb[:, c, :])
            sqs.append(sqc)

        def fin(s=s, bt=bt, t_sb=t_sb, sqs=sqs):
            st_sum = st_ps.tile([1, 512], F32, name="st", tag="stps")
            for c in range(4):
                nc.tensor.matmul(st_sum[:], ones_col[:], t_sb[:, c, :],
                                 start=(c == 0), stop=(c == 3))
            st_sq = stats(sqs)
            mu = rowf.tile([1, 512], F32, name="mu", tag="rowf")
            nc.scalar.activation(mu[:], st_sum[:], ACT.Copy, scale=1.0 / D)
            m2 = rowf.tile([1, 512], F32, name="m2", tag="rowf")
            nc.vector.tensor_mul(m2[:], mu[:], mu[:])
            v = rowf.tile([1, 512], F32, name="vv", tag="rowf")
            nc.vector.scalar_tensor_tensor(v[:], m2[:], -float(D), st_sq[:],
                                           ALU.mult, ALU.add)
            rec = rowf.tile([1, 512], F32, name="rec", tag="rowf")
            nc.vector.reciprocal_approx_fast(rec[:], v[:])
            ist = rowi.tile([1, 512], BF, name="ist", tag="rowi")
            nc.scalar.activation(ist[:], rec[:], ACT.Sqrt, scale=float(D))
            wr = wrp.tile([1, 512], BF, name="wr1")
            nc.vector.tensor_mul(wr[:], mu[:], ist[:])
            wrow[bt][s] = wr
            bc = bcp.tile([128, 512], BF, name="bct")
            nc.gpsimd.partition_broadcast(bc[:], ist[0:1, :])
            zt_sb = big.tile([128, 4, 512], BF, name="big_sb")
            for c in range(4):
                nc.vector.tensor_mul(zt_sb[:, c, :], t_sb[:, c, :], bc[:])
            zt[s][bt] = zt_sb
        pend.append(fin)

    def f1_group(bt):
        if bt == 0:
            f1lw = [load_pairs(io["w_f1l"][s], 2) for s in range(3)]
            f1ww = [load_pairs(io["w_f1"][s], 4) for s in range(3)]
            f1w[0] = (f1lw, f1ww)
        f1lw, f1ww = f1w[0]
        ps = [mm_ps.tile([128, 512], F32, name="mm") for _ in range(4)]
        seq = []
        for s in range(3):
            seq += [(f1lw[s][c], l_[s][bt][:, c, :]) for c in range(2)]
        for s in (2, 1, 0):
            seq += [(f1ww[s][c], zt[s][bt][:, c, :]) for c in range(4)]
        for s in (2, 1, 0):
            seq.append((lambda m, s=s: negc3[0:1, s, ts(m, 128)],
                        wrow[bt][s][:]))
        last = len(seq) - 1
        for ci, (lf, rhs) in enumerate(seq):
            for m in range(4):
                nc.tensor.matmul(ps[m][:], lf(m), rhs,
                                 start=(ci == 0), stop=(ci == last))
        ev, sq = evict_sq(ps, [b_f1[:, c:c + 1] for c in range(4)])

        def fin(bt=bt, ev=ev, sq=sq):
            st = stats(sq)
            h_sb = big.tile([128, 4, 512], BF, name="big_sb")
            ln_finish(ev, st, D, g_f1, be_f1, ACT.Gelu, h_sb)
            h_[bt] = h_sb
        pend.append(fin)

    def f2_group(bt):
        bsl = ts(bt, 512)
        if bt == 0:
            f2w[0] = load_pairs(io["w_f2"], 4)
        ps = emit_mms(f2w[0], lambda c: h_[bt][:, c, :])
        ev, sq = evict_sq(ps, [b_f2[:, c:c + 1] for c in range(4)])

        def fin(bt=bt, bsl=bsl, ev=ev, sq=sq):
            st = stats(sq)
            bc = istd_chain(st, D)
            for c, et in enumerate(ev):
                z = op_.tile([128, 512], F32, name="ot")
                nc.vector.tensor_mul(z[:], et[:], bc[:])
                o = op_.tile([128, 512], F32, name="ot")
                nc.scalar.activation(o[:], z[:], ACT.Identity,
                                     bias=be_f2[:, c:c + 1],
                                     scale=g_f2[:, c:c + 1])
                nc.sync.dma_start(io["outT"][ts(c, 128), bsl], o[:])
        pend.append(fin)

    # ---------------- emission schedule ----------------
    for s in range(3):
        for bt in range(2):
            lp_group(s, bt)
    hp_group(0, 0); hp_group(0, 1)
    flush(6)                      # lp fins (under hp0 matmuls)
    hp_group(1, 0); hp_group(1, 1)
    flush(2)                      # hp0 fins
    hp_group(2, 0); hp_group(2, 1)
    flush(2)                      # hp1 fins
    r_group(0, 0); r_group(0, 1)
    flush(2)                      # hp2 fins
    r_group(1, 0); r_group(1, 1)
    flush(2)                      # r0 fins -> e0
    r_group(2, 0); r_group(2, 1)
    flush(2)                      # r1 fins -> e1
    m_group(2, 0); m_group(2, 1)  # needs e0, e1
    flush(2)                      # r2 fins -> e2
    g_group(2, 0); g_group(2, 1)
    m_group(1, 0); m_group(1, 1)  # needs e0, e2
    flush(2)                      # gate2 fins -> zt2 (under m1 matmuls)
    g_group(1, 0); g_group(1, 1)
    m_group(0, 0); m_group(0, 1)  # needs e1, e2
    flush(2)                      # gate1 fins -> zt1
    g_group(0, 0); g_group(0, 1)
    flush(2)                      # gate0 fins -> zt0
    f1_group(0)
    f1_group(1)
    flush(1)                      # f1_0 fin -> h0 (under f1_1 matmuls)
    f2_group(0)
    flush(1)                      # f1_1 fin -> h1
    f2_group(1)
    flush()                       # f2 fins -> out DMAs
    ctx.close()


def build_program():
    nc = bacc.Bacc("TRN2", target_bir_lowering=False, debug=False,
                   num_devices=NCORES)
    io = {}

    def din(name, shape, dtype=F32):
        io[name] = nc.dram_tensor(name, list(shape), dtype,
                                  kind="ExternalInput").ap()

    for s in range(3):
        din(f"xT{s}", (HID, BL), dtype=BF)
        din(f"lT{s}", (FS[s], BL))
    din("w_hp", (3, 8, 128, 512), dtype=BF)
    din("b_hp", (3, 128, 4))
    din("w_r", (4, 128, 512), dtype=BF)
    din("b_r", (128, 4))
    din("w_m", (3, 8, 128, 512), dtype=BF)
    din("b_m", (3, 128, 4))
    din("w_g", (3, 8, 128, 512), dtype=BF)
    for s in range(3):
        din(f"w_lp{s}", (FS[s], 256), dtype=BF)
    din("b_lp", (3, 128, 2))
    din("w_f1", (3, 4, 128, 512), dtype=BF)
    din("w_f1l", (3, 2, 128, 512), dtype=BF)
    din("negc_f1", (1, 3, 512), dtype=BF)
    din("b_f1", (128, 4))
    din("w_f2", (4, 128, 512), dtype=BF)
    din("b_f2", (128, 4))
    for name in ("g_hp", "be_hp", "g_n1", "be_n1", "b_g"):
        din(name, (3, 128, 4))
    for name in ("g_lp", "be_lp"):
        din(name, (3, 128, 2))
    for name in ("g_f1", "be_f1", "g_f2", "be_f2"):
        din(name, (128, 4))
    din("ones_col", (128, 1), dtype=BF)
    din("eps_lhs", (1, 1), dtype=BF)
    din("one_row", (1, 512), dtype=BF)
    io["outT"] = nc.dram_tensor("outT", [D, BL], F32,
                                kind="ExternalOutput").ap()

    with tile.TileContext(nc) as tc:
        emit_program(tc, io)
    nc.compile()
    return nc


def make_in_maps(inputs):
    fw = fold_weights(inputs)
    dev = device_arrays(fw)
    hidden = [np.asarray(inputs["verb_hidden"], np.float32),
              np.asarray(inputs["inst_hidden"], np.float32),
              np.asarray(inputs["target_hidden"], np.float32)]
    logits = [np.asarray(inputs["verb_logits"], np.float32),
              np.asarray(inputs["inst_logits"], np.float32),
              np.asarray(inputs["target_logits"], np.float32)]
    in_maps = []
    for core in range(NCORES):
        rows = slice(core * BL, (core + 1) * BL)
        m = dict(dev)
        for s in range(3):
            m[f"xT{s}"] = np.ascontiguousarray(
                hidden[s][rows].T.astype(BF_NP))
            m[f"lT{s}"] = np.ascontiguousarray(logits[s][rows].T)
        in_maps.append(m)
    return in_maps


_NC_CACHE = None


def _run(inputs, **spmd_kwargs):
    global _NC_CACHE
    if _NC_CACHE is None:
        _NC_CACHE = build_program()
    nc = _NC_CACHE
    in_maps = make_in_maps(inputs)
    res = run_bass_kernel_spmd(nc, in_maps, list(range(NCORES)),
                               **spmd_kwargs)
    out = np.empty((B, D), dtype=np.float32)
    for core in range(NCORES):
        out[core * BL:(core + 1) * BL] = res.results[core]["outT"].T
    return out, res


def kernel(**inputs) -> np.ndarray:
    return _run(inputs)[0]


def kernel_profiled(inputs, tmpdir=None):
    """Returns (out, BassKernelResults) with an NTFF-based profile."""
    return _run(inputs, trace=True, tmpdir=tmpdir)
